# revision 1
# baseline (speedup 1.0000x reference)
"""ChaosNet (ChaosFEX + linear head) Trainium2 kernel.

Math restructure: every per-element feature depends only on k*(x) = first
trajectory index k with |traj[k] - x| < eps.  k*(x) is piecewise-constant in x
(first-claim intervals of the shared trajectory), so the model output

    out[n, c] = b_c + sum_f Phi_{c,f}(k*(x[n,f]))

is, per (c, f), a piecewise-constant function of x with M segments.  With
region left-edges L_0 <= ... <= L_{M-1} and per-segment table values Phi[m],
a telescoped form needs only rank indicators:

    Phi(x) = sum_m [x >= L_m] * dPhi[m]          (dPhi = successive deltas)

On device (per core, 256 rows of x):
  - gpsimd broadcasts x (f-major [1, 8192]) across 128 partitions
  - vector engine computes u[m, j] = (x[j] >= L[m])  in fp32 {0,1}
  - tensor engine accumulates out[c, n] += sum_m u[m, f*256+n] * dPhi[m, 2f+c]
    over all 32 f-blocks into one [2, 256] PSUM tile
  - bias add, DMA out.

The host does only the inherently sequential scalar work: the 10000-step
trajectory, its prefix sums, and the exact-fp32 region partition (binary
search on fp32 bit patterns, so region edges reproduce the reference's
fp32 comparison semantics exactly).
"""

import os
import sys
from contextlib import ExitStack

import numpy as np

sys.path.insert(0, "/opt/trn_rl_repo")

import concourse.bass as bass  # noqa: E402
import concourse.tile as tile  # noqa: E402
from concourse import bacc, mybir  # noqa: E402
from concourse.bass_utils import run_bass_kernel_spmd  # noqa: E402

T = 10000
N = 2048
F = 32
NCORES = 8
N_LOC = N // NCORES            # 256 rows per core
E = N_LOC * F                  # 8192 elements per core
MCHUNK = 128                   # region-table rows per partition chunk
SLICE = 1024                   # free-dim pipeline slice (4 f-blocks)

np.seterr(all="ignore")

LAST_RESULTS = None            # BassKernelResults of the most recent run
LAST_NC = None                 # compiled Bass program of the most recent run


# ----------------------------------------------------------------------------
# Host-side preprocessing
# ----------------------------------------------------------------------------

def _build_traj(ic, thr):
    """fp32 skew-tent trajectory, bit-identical to the jax scan."""
    traj = np.empty(T, np.float32)
    z = np.float32(ic)
    thr = np.float32(thr)
    one = np.float32(1.0)
    omt = np.float32(one - thr)
    for k in range(T):
        traj[k] = z
        z = np.float32(z / thr) if z < thr else np.float32((one - z) / omt)
    return traj


def _sortable(i):
    """int32 bit pattern -> order-isomorphic int32 key (handles negatives)."""
    return np.where(i >= 0, i, i ^ np.int32(0x7FFFFFFF))


def _unsortable(k):
    return np.where(k >= 0, k, k ^ np.int32(0x7FFFFFFF))


def _match_intervals(traj, eps, xmin, xmax):
    """Exact fp32 interval [lo_k, hi_k] of {x in [xmin,xmax] :
    |fl32(traj_k - x)| < eps}; valid[k]=False if empty."""
    eps = np.float32(eps)
    xmin = np.float32(xmin)
    xmax = np.float32(xmax)

    def cond(xs):
        return np.abs(traj - xs.astype(np.float32)) < eps

    anchor = np.clip(traj, xmin, xmax)
    valid = cond(anchor)

    I = lambda f: _sortable(f.view(np.int32))             # noqa: E731
    Fv = lambda k: _unsortable(k).view(np.float32)        # noqa: E731

    def bisect(lo_i, hi_i, need, want_smallest_true):
        # invariant: cond(Fv(hi_i)) True/False per direction; int keys.
        for _ in range(40):
            gap = np.where(need, hi_i - lo_i, 0)
            if (gap <= 1).all():
                break
            mid = ((lo_i.astype(np.int64) + hi_i) // 2).astype(np.int32)
            cm = cond(Fv(mid))
            if want_smallest_true:
                hi_i = np.where(need & cm, mid, hi_i)
                lo_i = np.where(need & ~cm, mid, lo_i)
            else:
                lo_i = np.where(need & cm, mid, lo_i)
                hi_i = np.where(need & ~cm, mid, hi_i)
        return lo_i, hi_i

    # left edge: smallest x in [xmin, anchor] with cond True
    at_min = cond(np.full(T, xmin, np.float32))
    lo_edge = np.where(at_min, xmin, np.float32(np.nan))
    need = valid & np.isnan(lo_edge)
    lo_i = np.broadcast_to(I(xmin.reshape(1)), (T,)).copy()
    hi_i = I(anchor.copy())
    lo_i, hi_i = bisect(lo_i, hi_i, need, True)
    lo_edge = np.where(np.isnan(lo_edge), Fv(hi_i), lo_edge)

    # right edge: largest x in [anchor, xmax] with cond True
    at_max = cond(np.full(T, xmax, np.float32))
    hi_edge = np.where(at_max, xmax, np.float32(np.nan))
    need = valid & np.isnan(hi_edge)
    lo_i = I(anchor.copy())
    hi_i = np.broadcast_to(I(xmax.reshape(1)), (T,)).copy()
    lo_i, hi_i = bisect(lo_i, hi_i, need, False)
    hi_edge = np.where(np.isnan(hi_edge), Fv(lo_i), hi_edge)

    # exactness checks (cheap, vectorized)
    v = valid
    assert cond(np.where(v, lo_edge, anchor)).all()
    assert cond(np.where(v, hi_edge, anchor)).all()
    below = np.nextafter(lo_edge, np.float32(-np.inf))
    above = np.nextafter(hi_edge, np.float32(np.inf))
    assert not (v & (below >= xmin) & cond(below)).any()
    assert not (v & (above <= xmax) & cond(above)).any()
    return lo_edge, hi_edge, valid


def _build_regions(traj, eps, xmin, xmax):
    """First-claim partition of [xmin, xmax] into regions of constant k*.
    Returns sorted left edges L (fp32) and per-region kstar (== T: never)."""
    xl, xr, valid = _match_intervals(traj, eps, xmin, xmax)
    down = lambda a: np.nextafter(a, np.float32(-np.inf))  # noqa: E731
    up = lambda a: np.nextafter(a, np.float32(np.inf))     # noqa: E731
    uncovered = [(np.float32(xmin), np.float32(xmax))]
    regions = []
    for k in range(T):
        if not uncovered:
            break
        if not valid[k]:
            continue
        lo_k, hi_k = xl[k], xr[k]
        new_unc = []
        for (a, b) in uncovered:
            if lo_k > b or hi_k < a:
                new_unc.append((a, b))
                continue
            ra, rb = max(lo_k, a), min(hi_k, b)
            regions.append((ra, k))
            if a < ra:
                new_unc.append((a, down(ra)))
            if rb < b:
                new_unc.append((up(rb), b))
        uncovered = new_unc
    for (a, b) in uncovered:
        regions.append((a, T))
    regions.sort(key=lambda r: r[0])
    L = np.array([r[0] for r in regions], np.float32)
    ks = np.array([r[1] for r in regions], np.int64)
    return L, ks


def _region_features(traj, thr, ks):
    """Per-region (tt, energy, p, ent) with the reference's fp32 accumulation
    semantics (sequential fp32 cumsum == per-step fp32 adds)."""
    thr = np.float32(thr)
    t2 = traj * traj                                  # fp32 squares
    Ecum = np.cumsum(t2, dtype=np.float32)            # sequential fp32 adds
    gt = (traj > thr).astype(np.float32)
    Ccum = np.cumsum(gt, dtype=np.float32)            # exact small ints

    fired = ks < T
    j = np.where(fired, ks, T - 1)
    tt = np.where(fired, ks + 1, T).astype(np.float32)
    en = Ecum[j].astype(np.float32)
    cnt = Ccum[j].astype(np.float32)
    p = (cnt / tt).astype(np.float32)

    def xlog2x(v):
        safe = np.where(v > 0, v, np.float32(1.0)).astype(np.float32)
        return np.where(v > 0, v * np.log2(safe, dtype=np.float32),
                        np.float32(0.0)).astype(np.float32)

    ent = -(xlog2x(p) + xlog2x((np.float32(1.0) - p).astype(np.float32)))
    return tt, en, p, ent.astype(np.float32)


def _build_tables(x, ic, thr, eps, W, b):
    """Builds all device-side tables.  Row split: the DVE path
    (scalar_tensor_tensor telescoping) covers n < 4*q_dve; the PE path
    (fp16 hi/lo pair matmuls over rank indicators) covers the rest, for
    all 32 features."""
    traj = _build_traj(ic, thr)
    L, ks = _build_regions(traj, eps, float(x.min()), float(x.max()))
    tt, en, p, ent = _region_features(traj, thr, ks)
    M = L.shape[0]

    # Phi[m, 2f+c] = W[c,4f]*tt + W[c,4f+1]*en + W[c,4f+2]*p + W[c,4f+3]*ent
    W64 = W.astype(np.float64).reshape(2, F, 4)
    feats64 = np.stack([tt, en, p, ent], -1).astype(np.float64)   # [M, 4]
    phi = np.einsum("mj,cfj->mcf", feats64, W64)                  # [M, 2, F]
    phi = phi.transpose(0, 2, 1).reshape(M, 2 * F)                # [M, 64]

    # compensated fp32 deltas: partial fp32 sums track the fp64 table
    dphi = np.empty((M, 2 * F), np.float32)
    running = np.zeros(2 * F, np.float64)
    for m in range(M):
        d = (phi[m] - running).astype(np.float32)
        dphi[m] = d
        running += d.astype(np.float64)

    # pad M to a multiple of 32 free-dim elements; L pad = +inf (never <= x)
    mp = max(32, ((M + 31) // 32) * 32)
    L_pad = np.full(mp, np.float32(np.inf), np.float32)
    L_pad[:M] = L
    dphi_pad = np.zeros((mp, 2 * F), np.float32)
    dphi_pad[:M] = dphi
    per_f = dphi_pad.reshape(mp, F, 2)                 # [m, f, c]

    # ---- DVE-path layouts (partition p = f + 32*r, r = n % 4) ----
    #   lb   [128, mp]      L replicated across partitions
    #   dstt [2, 128, mp]   dstt[c][f+32r, m] = dPhi_{c,f}[m]
    #   s8   [2, 128, 8]    reduction stationary: S_c[f+32r, c+2r] = 1
    #   bias8 [8, 1]        bias8[c+2r] = b[c]
    lb = np.broadcast_to(L_pad, (128, mp)).copy()
    dstt = np.zeros((2, 128, mp), np.float32)
    s8 = np.zeros((2, 128, 8), np.float32)
    for c in range(2):
        for r in range(4):
            for f in range(F):
                dstt[c, f + 32 * r, :] = per_f[:, f, c]
                s8[c, f + 32 * r, c + 2 * r] = 1.0
    bias8 = np.empty((8, 1), np.float32)
    for r in range(4):
        for c in range(2):
            bias8[c + 2 * r, 0] = b[c]

    # ---- PE-path layouts (contraction over m, fp16 hi/lo pair) ----
    #   lpe  [mp, 1]       region edges down the partitions
    #   whi  [mp, 4*F]     stationary: cols 4f.. = (hi_c0, hi_c1, lo_c0, lo_c1)
    lpe = L_pad.reshape(mp, 1).copy()
    hi16 = per_f.reshape(mp, 2 * F).astype(np.float16)
    lo16 = (per_f.reshape(mp, 2 * F).astype(np.float64)
            - hi16.astype(np.float64)).astype(np.float16)
    whi = np.empty((mp, 4 * F), np.float16)
    for f in range(F):
        whi[:, 4 * f:4 * f + 2] = hi16[:, 2 * f:2 * f + 2]
        whi[:, 4 * f + 2:4 * f + 4] = lo16[:, 2 * f:2 * f + 2]
    return lb, dstt, s8, bias8, lpe, whi, mp


# ----------------------------------------------------------------------------
# Device kernel
# ----------------------------------------------------------------------------

NCOL = N_LOC // 4              # 64 element-columns of 128 per core


def _build_device_program(mp, q):
    """q = columns (of 128 elements) on the DVE path; rows n >= 4q go to
    the PE path for all F features."""
    npe = N_LOC - 4 * q         # PE-path rows
    nc = bacc.Bacc("TRN2", target_bir_lowering=False, debug=False,
                   num_devices=NCORES)
    f32 = mybir.dt.float32
    f16 = mybir.dt.float16
    is_le = mybir.AluOpType.is_le
    is_ge = mybir.AluOpType.is_ge
    mult = mybir.AluOpType.mult
    add = mybir.AluOpType.add

    if q:
        xc_d = nc.dram_tensor("xc", [128, q], f32, kind="ExternalInput").ap()
        lb_d = nc.dram_tensor("lb", [128, mp], f32, kind="ExternalInput").ap()
        d0_d = nc.dram_tensor("d0", [128, mp], f32, kind="ExternalInput").ap()
        d1_d = nc.dram_tensor("d1", [128, mp], f32, kind="ExternalInput").ap()
        s8_d = nc.dram_tensor("s8", [2, 128, 8], f32,
                              kind="ExternalInput").ap()
        bias_d = nc.dram_tensor("bias", [8, 1], f32,
                                kind="ExternalInput").ap()
        out_d = nc.dram_tensor("out", [8, q], f32, kind="ExternalOutput").ap()
    if npe:
        epe = F * npe           # elements on the PE path
        xf_d = nc.dram_tensor("xf", [1, epe], f32, kind="ExternalInput").ap()
        lpe_d = nc.dram_tensor("lpe", [mp, 1], f32, kind="ExternalInput").ap()
        whi_d = nc.dram_tensor("whi", [mp, 4 * F], f16,
                               kind="ExternalInput").ap()
        ope_d = nc.dram_tensor("outpe", [4, npe], f32,
                               kind="ExternalOutput").ap()

    n_slice = 2
    with tile.TileContext(nc) as tc, ExitStack() as ctx:
        consts = ctx.enter_context(tc.tile_pool(name="consts", bufs=1))
        scr = ctx.enter_context(tc.tile_pool(name="scr", bufs=1))
        gp = ctx.enter_context(tc.tile_pool(name="g", bufs=1))
        outp = ctx.enter_context(tc.tile_pool(name="outp", bufs=1))
        psum = ctx.enter_context(tc.tile_pool(name="psum", bufs=2,
                                              space="PSUM"))

        # spread input DMAs over the two HWDGE queues (SP + Activation),
        # compute dependencies first
        if npe:
            xf = consts.tile([1, epe], f32, tag="xf")
            nc.scalar.dma_start(xf[:, :], xf_d)
            lpe = consts.tile([mp, 1], f32, tag="lpe")
            nc.scalar.dma_start(lpe[:, :], lpe_d)
            whi = consts.tile([mp, 4 * F], f16, tag="whi")
            nc.scalar.dma_start(whi[:, :], whi_d)
        if q:
            xc = consts.tile([128, q], f32, tag="xc")
            nc.sync.dma_start(xc[:, :], xc_d)
            lb = consts.tile([128, mp], f32, tag="lb")
            nc.sync.dma_start(lb[:, :], lb_d)
            dstt = [consts.tile([128, mp], f32, tag=f"d{c}", name=f"dstt{c}")
                    for c in range(2)]
            nc.sync.dma_start(dstt[0][:, :], d0_d)
            nc.sync.dma_start(dstt[1][:, :], d1_d)
            s8 = [consts.tile([128, 8], f32, tag=f"s8_{c}", name=f"s8t{c}")
                  for c in range(2)]
            for c in range(2):
                nc.scalar.dma_start(s8[c][:, :], s8_d[c])
            bias = consts.tile([8, 1], f32, tag="bias")
            nc.scalar.dma_start(bias[:, :], bias_d)

        # ---- PE path: broadcast x, compare to region edges, fp16 matmuls
        if npe:
            accpe = psum.tile([4, npe], f32, tag="accpe")
            xb = gp.tile([mp, epe], f32, tag="xb")
            u16 = gp.tile([mp, epe], f16, tag="u16")
            f_per = [F // n_slice + (1 if s < F % n_slice else 0)
                     for s in range(n_slice)]
            f0 = 0
            for s in range(n_slice):
                sl = slice(f0 * npe, (f0 + f_per[s]) * npe)
                nc.gpsimd.partition_broadcast(xb[:, sl], xf[:, sl])
                nc.vector.tensor_scalar(u16[:, sl], xb[:, sl], lpe[:, :],
                                        None, is_ge)
                f0 += f_per[s]
            for f in range(F):
                usl = u16[:, f * npe:(f + 1) * npe]
                nc.tensor.matmul(accpe[:, :], whi[:, 4 * f:4 * f + 4], usl,
                                 start=(f == 0), stop=(f == F - 1))
            outpe = outp.tile([4, npe], f32, tag="outpe")
            nc.scalar.mul(outpe[:, :], accpe[:, :], 1.0)
            nc.scalar.dma_start(ope_d, outpe[:, :])

        # ---- DVE path: per-column telescoped rank sums for n < 4q
        if q:
            g = {}
            scratch = {}
            for c in range(2):
                g["v", c] = gp.tile([128, q], f32, tag=f"gv{c}",
                                    name=f"gv{c}")
                scratch["v", c] = scr.tile([128, mp], f32, tag=f"sv{c}",
                                           name=f"sv{c}")
            for c in range(2):
                for col in range(q):
                    xs = xc[:, col:col + 1]
                    nc.vector.scalar_tensor_tensor(
                        scratch["v", c][:, :], lb[:, :], xs, dstt[c][:, :],
                        is_le, mult, accum_out=g["v", c][:, col:col + 1])

            acc = psum.tile([8, q], f32, tag="acc8")
            nc.tensor.matmul(acc[:, :], s8[0][:, :], g["v", 0][:, :],
                             start=True, stop=False)
            nc.tensor.matmul(acc[:, :], s8[1][:, :], g["v", 1][:, :],
                             start=False, stop=True)

            outs = outp.tile([8, q], f32)
            nc.vector.tensor_scalar(outs[:, :], acc[:, :], bias[:, :],
                                    None, add)
            nc.sync.dma_start(out_d, outs[:, :])

    nc.compile()
    return nc


# ----------------------------------------------------------------------------
# Entry point
# ----------------------------------------------------------------------------

def kernel(x, initial_cond, threshold, epsilon, W, b):
    global LAST_RESULTS, LAST_NC
    x = np.ascontiguousarray(np.asarray(x, np.float32))
    W = np.asarray(W, np.float32)
    b = np.asarray(b, np.float32)
    ic = float(np.asarray(initial_cond).reshape(-1)[0])
    thr = float(np.asarray(threshold).reshape(-1)[0])
    eps = float(np.asarray(epsilon).reshape(-1)[0])

    q = int(os.environ.get("Q_DVE", "12"))
    npe = N_LOC - 4 * q
    lb, dstt, s8, bias8, lpe, whi, mp = _build_tables(x, ic, thr, eps, W, b)

    nc = _build_device_program(mp, q)
    LAST_NC = nc

    in_maps = []
    for d in range(NCORES):
        xd = x[d * N_LOC:(d + 1) * N_LOC, :]            # [256, 32]
        im = {}
        if q:
            # xc[f + 32r, col] = x[4*col + r, f]  for n < 4q
            im.update({
                "xc": np.ascontiguousarray(
                    xd[:4 * q].reshape(q, 4, F).transpose(1, 2, 0)
                    .reshape(128, q)),
                "lb": lb, "d0": dstt[0], "d1": dstt[1],
                "s8": s8, "bias": bias8,
            })
        if npe:
            # f-major x for the PE path: xf[0, f*npe + j] = x[4q + j, f]
            im.update({
                "xf": np.ascontiguousarray(xd[4 * q:].T).reshape(1, F * npe),
                "lpe": lpe, "whi": whi,
            })
        in_maps.append(im)

    res = run_bass_kernel_spmd(nc, in_maps, core_ids=list(range(NCORES)))
    LAST_RESULTS = res

    out = np.empty((N, 2), np.float32)
    for d in range(NCORES):
        row0 = d * N_LOC
        if q:
            o8 = res.results[d]["out"]                  # [8, q]: [c+2r, col]
            out[row0:row0 + 4 * q, :] = (
                o8.reshape(4, 2, q).transpose(2, 0, 1).reshape(4 * q, 2))
        if npe:
            o4 = res.results[d]["outpe"]                # [4, npe] hi/lo rows
            out[row0 + 4 * q:row0 + N_LOC, :] = (
                (o4[:2] + o4[2:]).T + b.reshape(1, 2))
    return out



# revision 6
# speedup vs baseline: 1.3053x; 1.3053x over previous
"""ChaosNet (ChaosFEX + linear head) Trainium2 kernel.

Math restructure: every per-element feature depends only on k*(x) = first
trajectory index k with |traj[k] - x| < eps.  k*(x) is piecewise-constant in x
(first-claim intervals of the shared trajectory), so the model output

    out[n, c] = b_c + sum_f Phi_{c,f}(k*(x[n,f]))

is, per (c, f), a piecewise-constant function of x with M segments.  With
region left-edges L_0 <= ... <= L_{M-1} and per-segment table values Phi[m],
a telescoped form needs only rank indicators:

    Phi(x) = sum_m [x >= L_m] * dPhi[m]          (dPhi = successive deltas)

Device pipeline (per core, 256 rows of x, E = 8192 elements, mp regions in
the partition dim):
  - x is broadcast across the mp partitions in 512-column chunks, two ways:
      * PE: one ones-matmul per chunk over an exact 3-way bf16 split of x
        (x = hi+mid+lo exactly; the [3,mp] all-ones stationary reproduces x
        bit-exactly in PSUM at 1 PE cycle/column), or
      * gpsimd partition_broadcast from an f-major fp32 row (slower per
        column but runs on an otherwise idle engine).
  - compares u[m, j] = [x_j >= L_m] run on two engines:
      * DVE tensor_scalar is_ge -> fp16 {0,1}  (exact), or
      * Activation Sign(x - L) -> fp16 {-1,0,+1}; the (s+1)/2 re-encoding is
        folded into 0.5-scaled weight tables plus a per-channel constant,
        and the (measure-zero) x == L exact hits are patched on the host.
  - PE accumulates out[c, n] += sum_m u[m, f*256+n] * W'[m, 2f+c] over all 32
    f-blocks into one [4, 256] PSUM tile (fp16 hi/lo weight pairs).
  - DVE copies PSUM -> SBUF, one DMA out.

The host does only the inherently sequential scalar work: the 10000-step
trajectory, its prefix sums, and the exact-fp32 region partition (binary
search on fp32 bit patterns, so region edges reproduce the reference's
fp32 comparison semantics exactly).
"""

import os
import sys
from contextlib import ExitStack

import ml_dtypes
import numpy as np

sys.path.insert(0, "/opt/trn_rl_repo")

import concourse.bass as bass  # noqa: E402
import concourse.tile as tile  # noqa: E402
from concourse import bacc, mybir  # noqa: E402
from concourse.bass_utils import run_bass_kernel_spmd  # noqa: E402

T = 10000
N = 2048
F = 32
NCORES = 8
N_LOC = N // NCORES            # 256 rows per core
E = N_LOC * F                  # 8192 elements per core (f-major columns)
CHUNK = 512                    # columns per pipeline chunk (= 2 f-blocks)
NCHUNK = E // CHUNK            # 16

np.seterr(all="ignore")

LAST_RESULTS = None            # BassKernelResults of the most recent run
LAST_NC = None                 # compiled Bass program of the most recent run


# ----------------------------------------------------------------------------
# Host-side preprocessing
# ----------------------------------------------------------------------------

def _build_traj(ic, thr):
    """fp32 skew-tent trajectory, bit-identical to the jax scan."""
    traj = np.empty(T, np.float32)
    z = np.float32(ic)
    thr = np.float32(thr)
    one = np.float32(1.0)
    omt = np.float32(one - thr)
    for k in range(T):
        traj[k] = z
        z = np.float32(z / thr) if z < thr else np.float32((one - z) / omt)
    return traj


def _sortable(i):
    """int32 bit pattern -> order-isomorphic int32 key (handles negatives)."""
    return np.where(i >= 0, i, i ^ np.int32(0x7FFFFFFF))


def _unsortable(k):
    return np.where(k >= 0, k, k ^ np.int32(0x7FFFFFFF))


def _match_intervals(traj, eps, xmin, xmax):
    """Exact fp32 interval [lo_k, hi_k] of {x in [xmin,xmax] :
    |fl32(traj_k - x)| < eps}; valid[k]=False if empty."""
    eps = np.float32(eps)
    xmin = np.float32(xmin)
    xmax = np.float32(xmax)

    def cond(xs):
        return np.abs(traj - xs.astype(np.float32)) < eps

    anchor = np.clip(traj, xmin, xmax)
    valid = cond(anchor)

    I = lambda f: _sortable(f.view(np.int32))             # noqa: E731
    Fv = lambda k: _unsortable(k).view(np.float32)        # noqa: E731

    def bisect(lo_i, hi_i, need, want_smallest_true):
        # invariant: cond(Fv(hi_i)) True/False per direction; int keys.
        for _ in range(40):
            gap = np.where(need, hi_i - lo_i, 0)
            if (gap <= 1).all():
                break
            mid = ((lo_i.astype(np.int64) + hi_i) // 2).astype(np.int32)
            cm = cond(Fv(mid))
            if want_smallest_true:
                hi_i = np.where(need & cm, mid, hi_i)
                lo_i = np.where(need & ~cm, mid, lo_i)
            else:
                lo_i = np.where(need & cm, mid, lo_i)
                hi_i = np.where(need & ~cm, mid, hi_i)
        return lo_i, hi_i

    # left edge: smallest x in [xmin, anchor] with cond True
    at_min = cond(np.full(T, xmin, np.float32))
    lo_edge = np.where(at_min, xmin, np.float32(np.nan))
    need = valid & np.isnan(lo_edge)
    lo_i = np.broadcast_to(I(xmin.reshape(1)), (T,)).copy()
    hi_i = I(anchor.copy())
    lo_i, hi_i = bisect(lo_i, hi_i, need, True)
    lo_edge = np.where(np.isnan(lo_edge), Fv(hi_i), lo_edge)

    # right edge: largest x in [anchor, xmax] with cond True
    at_max = cond(np.full(T, xmax, np.float32))
    hi_edge = np.where(at_max, xmax, np.float32(np.nan))
    need = valid & np.isnan(hi_edge)
    lo_i = I(anchor.copy())
    hi_i = np.broadcast_to(I(xmax.reshape(1)), (T,)).copy()
    lo_i, hi_i = bisect(lo_i, hi_i, need, False)
    hi_edge = np.where(np.isnan(hi_edge), Fv(lo_i), hi_edge)

    # exactness checks (cheap, vectorized)
    v = valid
    assert cond(np.where(v, lo_edge, anchor)).all()
    assert cond(np.where(v, hi_edge, anchor)).all()
    below = np.nextafter(lo_edge, np.float32(-np.inf))
    above = np.nextafter(hi_edge, np.float32(np.inf))
    assert not (v & (below >= xmin) & cond(below)).any()
    assert not (v & (above <= xmax) & cond(above)).any()
    return lo_edge, hi_edge, valid


def _build_regions(traj, eps, xmin, xmax):
    """First-claim partition of [xmin, xmax] into regions of constant k*.
    Returns sorted left edges L (fp32) and per-region kstar (== T: never)."""
    xl, xr, valid = _match_intervals(traj, eps, xmin, xmax)
    down = lambda a: np.nextafter(a, np.float32(-np.inf))  # noqa: E731
    up = lambda a: np.nextafter(a, np.float32(np.inf))     # noqa: E731
    uncovered = [(np.float32(xmin), np.float32(xmax))]
    regions = []
    for k in range(T):
        if not uncovered:
            break
        if not valid[k]:
            continue
        lo_k, hi_k = xl[k], xr[k]
        new_unc = []
        for (a, b) in uncovered:
            if lo_k > b or hi_k < a:
                new_unc.append((a, b))
                continue
            ra, rb = max(lo_k, a), min(hi_k, b)
            regions.append((ra, k))
            if a < ra:
                new_unc.append((a, down(ra)))
            if rb < b:
                new_unc.append((up(rb), b))
        uncovered = new_unc
    for (a, b) in uncovered:
        regions.append((a, T))
    regions.sort(key=lambda r: r[0])
    L = np.array([r[0] for r in regions], np.float32)
    ks = np.array([r[1] for r in regions], np.int64)
    return L, ks


def _region_features(traj, thr, ks):
    """Per-region (tt, energy, p, ent) with the reference's fp32 accumulation
    semantics (sequential fp32 cumsum == per-step fp32 adds)."""
    thr = np.float32(thr)
    t2 = traj * traj                                  # fp32 squares
    Ecum = np.cumsum(t2, dtype=np.float32)            # sequential fp32 adds
    gt = (traj > thr).astype(np.float32)
    Ccum = np.cumsum(gt, dtype=np.float32)            # exact small ints
    fired = ks < T
    j = np.where(fired, ks, T - 1)
    tt = np.where(fired, ks + 1, T).astype(np.float32)
    en = Ecum[j].astype(np.float32)
    cnt = Ccum[j].astype(np.float32)
    p = (cnt / tt).astype(np.float32)

    def xlog2x(v):
        safe = np.where(v > 0, v, np.float32(1.0)).astype(np.float32)
        return np.where(v > 0, v * np.log2(safe, dtype=np.float32),
                        np.float32(0.0)).astype(np.float32)

    ent = -(xlog2x(p) + xlog2x((np.float32(1.0) - p).astype(np.float32)))
    return tt, en, p, ent.astype(np.float32)


def _split_bf16_3(x32):
    """Exact 3-way bf16 split: x == hi + mid + lo (verified)."""
    bf = ml_dtypes.bfloat16
    hi = x32.astype(bf)
    r1 = (x32 - hi.astype(np.float32)).astype(np.float32)
    mid = r1.astype(bf)
    r2 = (r1 - mid.astype(np.float32)).astype(np.float32)
    lo = r2.astype(bf)
    recon = ((hi.astype(np.float32) + mid.astype(np.float32))
             + lo.astype(np.float32)).astype(np.float32)
    assert np.array_equal(recon, x32), "3-way bf16 split is not exact"
    recon2 = (hi.astype(np.float32)
              + (mid.astype(np.float32) + lo.astype(np.float32)))
    assert np.array_equal(recon2.astype(np.float32), x32), \
        "3-way bf16 split order-sensitive"
    return hi, mid, lo


# Per-chunk routing.  BCAST[k] in {"pe", "gps"}; CMP[k] in {"dve", "act"}.
# "act" chunks use the Sign encoding (0.5-scaled tables + constant).
def _routes():
    gps = os.environ.get("GPS_CHUNKS", "11,12,13,14,15")
    act = os.environ.get("ACT_CHUNKS", "0,1,3,5,7,9,10")
    gps = set(int(s) for s in gps.split(",") if s != "")
    act = set(int(s) for s in act.split(",") if s != "")
    bcast = ["gps" if k in gps else "pe" for k in range(NCHUNK)]
    cmp_ = ["act" if k in act else "dve" for k in range(NCHUNK)]
    return bcast, cmp_


def _build_tables(x, ic, thr, eps, W, b):
    """Builds all device-side tables plus host-side output corrections."""
    traj = _build_traj(ic, thr)
    L, ks = _build_regions(traj, eps, float(x.min()), float(x.max()))
    tt, en, p, ent = _region_features(traj, thr, ks)
    M = L.shape[0]
    assert M <= 128, f"region count {M} exceeds one partition block"

    # Phi[m, 2f+c] = W[c,4f]*tt + W[c,4f+1]*en + W[c,4f+2]*p + W[c,4f+3]*ent
    W64 = W.astype(np.float64).reshape(2, F, 4)
    feats64 = np.stack([tt, en, p, ent], -1).astype(np.float64)   # [M, 4]
    phi = np.einsum("mj,cfj->mcf", feats64, W64)                  # [M, 2, F]
    phi = phi.transpose(0, 2, 1).reshape(M, 2 * F)                # [M, 64]

    # compensated fp32 deltas: partial fp32 sums track the fp64 table
    dphi = np.empty((M, 2 * F), np.float32)
    running = np.zeros(2 * F, np.float64)
    for m in range(M):
        d = (phi[m] - running).astype(np.float32)
        dphi[m] = d
        running += d.astype(np.float64)

    # pad M to a multiple of 8 partitions; L pad = +inf (never <= x)
    mp = max(16, ((M + 7) // 8) * 8)
    L_pad = np.full(mp, np.float32(np.inf), np.float32)
    L_pad[:M] = L
    dphi_pad = np.zeros((mp, 2 * F), np.float32)
    dphi_pad[:M] = dphi

    def pack_hilo(d32):
        """[mp, 2F] fp32 -> [mp, 4F] fp16: per f (hi_c0, hi_c1, lo_c0, lo_c1)."""
        hi16 = d32.astype(np.float16)
        lo16 = (d32.astype(np.float64) - hi16.astype(np.float64)) \
            .astype(np.float16)
        out = np.empty((mp, 4 * F), np.float16)
        for f in range(F):
            out[:, 4 * f:4 * f + 2] = hi16[:, 2 * f:2 * f + 2]
            out[:, 4 * f + 2:4 * f + 4] = lo16[:, 2 * f:2 * f + 2]
        return out

    whi = pack_hilo(dphi_pad)                        # is_ge chunks
    whs = pack_hilo(0.5 * dphi_pad)                  # Sign chunks (0.5-scaled)

    # consts [mp, 130] fp32: col0 = L, col1 = -L, cols 2:66 = whi (f16 pairs
    # viewed as f32 words), cols 66:130 = whs
    consts = np.zeros((mp, 130), np.float32)
    consts[:, 0] = L_pad
    consts[:, 1] = -L_pad
    consts[:, 2:66] = whi.view(np.float32)
    consts[:, 66:130] = whs.view(np.float32)

    bcast_r, cmp_r = _routes()
    # Sign-path constant per channel: K_c = sum over sign-chunk features f of
    # sum_m [(0.5 d)_hi + (0.5 d)_lo]  (from the actual device fp16 tables)
    K = np.zeros(2, np.float64)
    sign_f = [f for f in range(F) if cmp_r[f // 2] == "act"]
    for f in sign_f:
        for c in range(2):
            K[c] += (whs[:, 4 * f + c].astype(np.float64).sum()
                     + whs[:, 4 * f + 2 + c].astype(np.float64).sum())

    # exact x == L hits on Sign-path features lose 0.5*dphi (sign(0) = 0)
    corrections = []                                 # (n, f, m) triples
    hit_rows, hit_fs = np.nonzero(np.isin(x, L[:M]))
    for n, f in zip(hit_rows, hit_fs):
        if cmp_r[f // 2] != "act":
            continue
        m = int(np.nonzero(L[:M] == x[n, f])[0][0])
        corrections.append((int(n), int(f), m))

    return consts, whi, whs, mp, K, corrections, bcast_r, cmp_r


# ----------------------------------------------------------------------------
# Device kernel
# ----------------------------------------------------------------------------

def _build_device_program(mp, bcast_r, cmp_r):
    nc = bacc.Bacc("TRN2", target_bir_lowering=False, debug=False,
                   num_devices=NCORES)
    f32 = mybir.dt.float32
    f16 = mybir.dt.float16
    bf16 = mybir.dt.bfloat16
    is_ge = mybir.AluOpType.is_ge
    Sign = mybir.ActivationFunctionType.Sign

    xs_d = nc.dram_tensor("xs", [3, E + mp], bf16, kind="ExternalInput").ap()
    ct_d = nc.dram_tensor("ct", [mp, 130], f32, kind="ExternalInput").ap()
    need_xf = any(r == "gps" for r in bcast_r)
    if need_xf:
        xf_d = nc.dram_tensor("xf", [1, E], f32, kind="ExternalInput").ap()
    out_d = nc.dram_tensor("out", [4, N_LOC], f32, kind="ExternalOutput").ap()

    with tile.TileContext(nc) as tc, ExitStack() as ctx:
        consts = ctx.enter_context(tc.tile_pool(name="consts", bufs=1))
        warmp = ctx.enter_context(tc.tile_pool(name="warm", bufs=1))
        gpb = ctx.enter_context(tc.tile_pool(name="gpb", bufs=2))
        u16p = ctx.enter_context(tc.tile_pool(name="u16", bufs=4))
        outp = ctx.enter_context(tc.tile_pool(name="outp", bufs=1))
        psum = ctx.enter_context(tc.tile_pool(name="psum", bufs=3,
                                              space="PSUM"))
        psacc = ctx.enter_context(tc.tile_pool(name="psacc", bufs=1,
                                               space="PSUM"))
        pswarm = ctx.enter_context(tc.tile_pool(name="pswarm", bufs=1,
                                                space="PSUM"))

        # ---- input DMAs (dispatch order = HWDGE order) -----------------
        xs = consts.tile([3, E + mp], bf16, tag="xs")
        nc.sync.dma_start(xs[:, :], xs_d)            # SP queue, fastest decode
        ct = consts.tile([mp, 130], f32, tag="ct")
        nc.scalar.dma_start(ct[:, :], ct_d)          # Activation queue
        if need_xf:
            xf = consts.tile([1, E], f32, tag="xf")
            nc.gpsimd.dma_start(xf[:, :], xf_d)      # SWDGE, Pool engine

        lpe = ct[:, 0:1]
        nlpe = ct[:, 1:2]
        whi = ct[:, 2:66].bitcast(f16)               # [mp, 128]
        whs = ct[:, 66:130].bitcast(f16)
        ones3 = xs[:, E:E + mp]                      # [3, mp] all-ones bf16

        # ---- warmup: act table load + PE pstate ramp during the DMA head
        wb = warmp.tile([3, 2], bf16, tag="wb")
        nc.gpsimd.memset(wb[:, :], 0.0)
        wf = warmp.tile([1, 2], f32, tag="wf")
        nc.gpsimd.memset(wf[:, :], 0.0)
        wo = warmp.tile([1, 2], f16, tag="wo")
        nc.scalar.activation(wo[:, :], wf[:, :], Sign, bias=0.0, scale=1.0)
        pw = pswarm.tile([2, 2], f32, tag="pw")
        nc.tensor.matmul(pw[:, :], wb[:, 0:2], wb[:, 0:2],
                         start=True, stop=True)

        # ---- pipeline ---------------------------------------------------
        xb_ps = {}                                   # chunk -> PSUM tile
        xb_sb = {}                                   # chunk -> SBUF tile
        u16 = {}
        acc = psacc.tile([4, N_LOC], f32, tag="acc")

        def emit_bcast(k):
            sl = slice(k * CHUNK, (k + 1) * CHUNK)
            if bcast_r[k] == "pe":
                t = psum.tile([mp, CHUNK], f32, tag="xb", name=f"xb{k}")
                nc.tensor.matmul(t[:, :], ones3[:, :], xs[:, sl],
                                 start=True, stop=True)
                xb_ps[k] = t
            else:
                t = gpb.tile([mp, CHUNK], f32, tag="xg", name=f"xg{k}")
                nc.gpsimd.partition_broadcast(t[:, :], xf[:, sl])
                xb_sb[k] = t

        def emit_cmp(k):
            src = xb_ps[k] if k in xb_ps else xb_sb[k]
            u = u16p.tile([mp, CHUNK], f16, tag="u", name=f"u{k}")
            if cmp_r[k] == "dve":
                nc.vector.tensor_scalar(u[:, :], src[:, :], lpe, None, is_ge)
            else:
                nc.scalar.activation(u[:, :], src[:, :], Sign,
                                     bias=nlpe, scale=1.0)
            u16[k] = u

        acc_started = [False]

        def emit_acc(k):
            tbl = whi if cmp_r[k] == "dve" else whs
            for half in range(2):
                f = 2 * k + half
                usl = u16[k][:, half * N_LOC:(half + 1) * N_LOC]
                last = (k == NCHUNK - 1 and half == 1)
                nc.tensor.matmul(acc[:, :], tbl[:, 4 * f:4 * f + 4], usl,
                                 start=not acc_started[0], stop=last)
                acc_started[0] = True

        pe_chunks = [k for k in range(NCHUNK) if bcast_r[k] == "pe"]
        gps_chunks = [k for k in range(NCHUNK) if bcast_r[k] == "gps"]
        # gpsimd broadcasts stream independently on the Pool engine
        for k in gps_chunks:
            emit_bcast(k)
        # PE: keep 3 broadcast chunks in flight ahead of the accumulates
        lookahead = 3
        for k in pe_chunks[:lookahead]:
            emit_bcast(k)
        nxt = lookahead
        for k in range(NCHUNK):
            emit_cmp(k)
            emit_acc(k)
            if bcast_r[k] == "pe" and nxt < len(pe_chunks):
                emit_bcast(pe_chunks[nxt])
                nxt += 1

        # ---- output -----------------------------------------------------
        outs = outp.tile([4, N_LOC], f32)
        nc.vector.tensor_scalar(outs[:, :], acc[:, :], 0.0, None,
                                mybir.AluOpType.add)
        nc.sync.dma_start(out_d, outs[:, :])

    nc.compile()
    return nc


# ----------------------------------------------------------------------------
# Entry point
# ----------------------------------------------------------------------------

def kernel(x, initial_cond, threshold, epsilon, W, b):
    global LAST_RESULTS, LAST_NC
    x = np.ascontiguousarray(np.asarray(x, np.float32))
    W = np.asarray(W, np.float32)
    b = np.asarray(b, np.float32)
    ic = float(np.asarray(initial_cond).reshape(-1)[0])
    thr = float(np.asarray(threshold).reshape(-1)[0])
    eps = float(np.asarray(epsilon).reshape(-1)[0])

    consts, whi, whs, mp, K, corrections, bcast_r, cmp_r = _build_tables(
        x, ic, thr, eps, W, b)
    need_xf = any(r == "gps" for r in bcast_r)

    nc = _build_device_program(mp, bcast_r, cmp_r)
    LAST_NC = nc

    bf = ml_dtypes.bfloat16
    in_maps = []
    for d in range(NCORES):
        xd = x[d * N_LOC:(d + 1) * N_LOC, :]         # [256, 32]
        xrow = np.ascontiguousarray(xd.T).reshape(E)  # f-major
        hi, mid, lo = _split_bf16_3(xrow)
        xsplit = np.ones((3, E + mp), bf)
        xsplit[0, :E] = hi
        xsplit[1, :E] = mid
        xsplit[2, :E] = lo
        im = {"xs": xsplit, "ct": consts}
        if need_xf:
            im["xf"] = xrow.reshape(1, E)
        in_maps.append(im)

    res = run_bass_kernel_spmd(nc, in_maps, core_ids=list(range(NCORES)))
    LAST_RESULTS = res

    out = np.empty((N, 2), np.float64)
    for d in range(NCORES):
        o4 = res.results[d]["out"].astype(np.float64)  # [4, 256] hi/lo rows
        out[d * N_LOC:(d + 1) * N_LOC, :] = (o4[:2] + o4[2:]).T
    out += b.astype(np.float64).reshape(1, 2) + K.reshape(1, 2)
    for (n, f, m) in corrections:
        for c in range(2):
            out[n, c] += (float(whs[m, 4 * f + c])
                          + float(whs[m, 4 * f + 2 + c]))
    return out.astype(np.float32)
